# revision 9
# baseline (speedup 1.0000x reference)
"""Trainium2 Bass kernel for nn_DenseOTPE (B=32, DIN=DOUT=1024, 8 cores).

Data-parallel over batch axis 0: each of the 8 NeuronCores processes 4
samples. Params W, b are replicated. No collectives needed (all outputs
are per-sample).

Math (closed form of the reference, per sample i):
    h      = x @ W + b
    u_new  = BETA*u + h
    s      = sigmoid(u_new - THR)
    sg     = s*(1-s)
    E_W2   = BETA^2*E_W + (1+BETA)*x[:,None]            (broadcast over DOUT)
    Rh_W2  = sg[None,:] * (BETA*Rh_W + E_W2)
    E_b2   = BETA^2*E_b + (1+BETA)
    Rh_b2  = sg * (BETA*Rh_b + E_b2)
    ratio  = LEAK*r ; r2 = ratio+1 ; ratio' = ratio/r2
    g_bar2 = ratio'*g_bar + (1-ratio')*BETA*sg
"""

import os
import sys
from contextlib import ExitStack

for _p in ("/opt/trn_rl_repo", "/root/.axon_site/_ro/trn_rl_repo"):
    if os.path.isdir(_p) and _p not in sys.path:
        sys.path.insert(0, _p)

import numpy as np

import concourse.bacc as bacc
import concourse.bass as bass
import concourse.tile as tile
from concourse import mybir
from concourse._compat import get_trn_type
from concourse.bass_utils import run_bass_kernel_spmd

BETA = 0.9
THR = 1.0
LEAK = 0.9
B, DIN, DOUT = 32, 1024, 1024
NCORES = 8
BSH = B // NCORES          # 4 samples per core
NK = DIN // 128            # 8 din chunks of 128 partitions
F32 = mybir.dt.float32
ALU = mybir.AluOpType
ACTF = mybir.ActivationFunctionType


def build_program():
    nc = bacc.Bacc(get_trn_type() or "TRN2", target_bir_lowering=False, debug=False)

    x = nc.dram_tensor("x", [BSH, DIN], F32, kind="ExternalInput").ap()
    W = nc.dram_tensor("W", [DIN, DOUT], F32, kind="ExternalInput").ap()
    b = nc.dram_tensor("b", [1, DOUT], F32, kind="ExternalInput").ap()
    u = nc.dram_tensor("u", [BSH, DOUT], F32, kind="ExternalInput").ap()
    EW = nc.dram_tensor("E_W", [BSH, DIN, DOUT], F32, kind="ExternalInput").ap()
    Eb = nc.dram_tensor("E_b", [BSH, DOUT], F32, kind="ExternalInput").ap()
    RW = nc.dram_tensor("Rh_W", [BSH, DIN, DOUT], F32, kind="ExternalInput").ap()
    Rb = nc.dram_tensor("Rh_b", [BSH, DOUT], F32, kind="ExternalInput").ap()
    gb = nc.dram_tensor("g_bar", [BSH, DOUT], F32, kind="ExternalInput").ap()
    r = nc.dram_tensor("r", [BSH, 1], F32, kind="ExternalInput").ap()

    s_o = nc.dram_tensor("s_out", [BSH, DOUT], F32, kind="ExternalOutput").ap()
    EW_o = nc.dram_tensor("E_W2", [BSH, DIN, DOUT], F32, kind="ExternalOutput").ap()
    Eb_o = nc.dram_tensor("E_b2", [BSH, DOUT], F32, kind="ExternalOutput").ap()
    RW_o = nc.dram_tensor("Rh_W2", [BSH, DIN, DOUT], F32, kind="ExternalOutput").ap()
    Rb_o = nc.dram_tensor("Rh_b2", [BSH, DOUT], F32, kind="ExternalOutput").ap()
    gb_o = nc.dram_tensor("g_bar2", [BSH, DOUT], F32, kind="ExternalOutput").ap()
    r_o = nc.dram_tensor("r2", [BSH, 1], F32, kind="ExternalOutput").ap()

    with tile.TileContext(nc) as tc, ExitStack() as ctx:
        small = ctx.enter_context(tc.tile_pool(name="small", bufs=1))
        const = ctx.enter_context(tc.tile_pool(name="const", bufs=1))
        wpool = ctx.enter_context(tc.tile_pool(name="wpool", bufs=1))
        psum = ctx.enter_context(tc.tile_pool(name="psum", bufs=2, space="PSUM"))
        big_in = ctx.enter_context(tc.tile_pool(name="big_in", bufs=3))
        big_out = ctx.enter_context(tc.tile_pool(name="big_out", bufs=3))

        # ---------------- prologue: small loads ----------------
        xs = small.tile([BSH, DIN], F32, tag="xs")
        nc.sync.dma_start(xs[:], x[:])
        us = small.tile([BSH, DOUT], F32, tag="us")
        nc.sync.dma_start(us[:], u[:])
        ebs = small.tile([BSH, DOUT], F32, tag="ebs")
        nc.sync.dma_start(ebs[:], Eb[:])
        rbs = small.tile([BSH, DOUT], F32, tag="rbs")
        nc.sync.dma_start(rbs[:], Rb[:])
        gbs = small.tile([BSH, DOUT], F32, tag="gbs")
        nc.sync.dma_start(gbs[:], gb[:])
        rs = small.tile([BSH, 1], F32, tag="rs")
        nc.sync.dma_start(rs[:], r[:])
        b_row = small.tile([1, DOUT], F32, tag="brow")
        nc.sync.dma_start(b_row[:], b[:])

        w_sb = wpool.tile([128, NK * DOUT], F32, tag="w")
        for k in range(NK):
            nc.sync.dma_start(w_sb[:, bass.ts(k, DOUT)], W[bass.ts(k, 128), :])

        # constants
        ones4 = const.tile([BSH, BSH], F32, tag="ones4")
        nc.vector.memset(ones4[:], 1.0)
        ident4 = const.tile([BSH, BSH], F32, tag="id4")
        nc.gpsimd.affine_select(
            ident4[:], ones4[:], pattern=[[1, BSH]], compare_op=ALU.is_equal,
            fill=0.0, base=0, channel_multiplier=-1,
        )
        bident4 = const.tile([BSH, BSH], F32, tag="bid4")
        nc.vector.tensor_scalar_mul(bident4[:], ident4[:], BETA)
        ones1 = const.tile([1, 128], F32, tag="ones1")
        nc.vector.memset(ones1[:], 1.0)
        negthr = const.tile([BSH, 1], F32, tag="negthr")
        nc.vector.memset(negthr[:], -THR)
        # sel[:, i*128:(i+1)*128]: (BSH,128) matrix with row i all-ones —
        # lhsT that both selects sample i's sg row and replicates it to 128
        # partitions in one matmul.
        ones_sel = const.tile([BSH, BSH * 128], F32, tag="ones_sel")
        nc.vector.memset(ones_sel[:], 1.0)
        sel = const.tile([BSH, BSH * 128], F32, tag="sel")
        nc.gpsimd.affine_select(
            sel[:], ones_sel[:], pattern=[[1, BSH], [0, 128]],
            compare_op=ALU.is_equal, fill=0.0, base=0, channel_multiplier=-1,
        )

        # x^T tiles: xt[:, k*BSH+i] = x[i, k*128+p]; via PE transpose matmul
        xt = const.tile([128, NK * BSH], F32, tag="xt")
        for k in range(NK):
            pt = psum.tile([128, BSH], F32, tag="pxt")
            nc.tensor.matmul(pt[:], xs[:, bass.ts(k, 128)], ident4[:],
                             start=True, stop=True)
            nc.vector.tensor_copy(xt[:, bass.ts(k, BSH)], pt[:])
        xc1 = const.tile([128, NK * BSH], F32, tag="xc1")
        nc.vector.tensor_scalar_mul(xc1[:], xt[:], 1.0 + BETA)

        # u_new = x@W + b + BETA*u accumulated in PSUM; s = sigmoid(u_new-THR)
        s_sb = small.tile([BSH, DOUT], F32, tag="s")
        sg = small.tile([BSH, DOUT], F32, tag="sg")
        for n in range(2):
            ph = psum.tile([BSH, 512], F32, tag="ph")
            for k in range(NK):
                nc.tensor.matmul(
                    ph[:], xt[:, bass.ts(k, BSH)],
                    w_sb[:, k * DOUT + n * 512: k * DOUT + n * 512 + 512],
                    start=(k == 0), stop=False)
            nc.tensor.matmul(ph[:], ones1[0:1, 0:BSH], b_row[0:1, bass.ts(n, 512)],
                             start=False, stop=False)
            nc.tensor.matmul(ph[:], bident4[:], us[:, bass.ts(n, 512)],
                             start=False, stop=True)
            nc.scalar.activation(s_sb[:, bass.ts(n, 512)], ph[:], ACTF.Sigmoid,
                                 bias=negthr[:, 0:1])
        nc.sync.dma_start(s_o[:], s_sb[:])

        # sg = s*(1-s)
        tmp1 = small.tile([BSH, DOUT], F32, tag="scr", bufs=4)
        nc.vector.tensor_scalar(tmp1[:], s_sb[:], -1.0, 1.0, ALU.mult, ALU.add)
        nc.vector.tensor_mul(sg[:], tmp1[:], s_sb[:])

        # sg replicated across 128 partitions, per sample
        sgrep = const.tile([128, BSH * DOUT], F32, tag="sgrep")
        for i in range(BSH):
            for n in range(2):
                pr = psum.tile([128, 512], F32, tag="pr")
                nc.tensor.matmul(pr[:], sel[:, bass.ts(i, 128)],
                                 sg[:, bass.ts(n, 512)], start=True, stop=True)
                nc.vector.tensor_copy(
                    sgrep[:, i * DOUT + n * 512: i * DOUT + n * 512 + 512], pr[:])

        # ---------------- small trace updates ----------------
        eb2 = small.tile([BSH, DOUT], F32, tag="scr", bufs=4)
        nc.vector.tensor_scalar(eb2[:], ebs[:], BETA * BETA, 1.0 + BETA,
                                ALU.mult, ALU.add)
        nc.gpsimd.dma_start(Eb_o[:], eb2[:])
        vb = small.tile([BSH, DOUT], F32, tag="scr", bufs=4)
        nc.vector.scalar_tensor_tensor(vb[:], rbs[:], BETA, eb2[:],
                                       ALU.mult, ALU.add)
        rb2 = small.tile([BSH, DOUT], F32, tag="scr", bufs=4)
        nc.vector.tensor_mul(rb2[:], vb[:], sg[:])
        nc.gpsimd.dma_start(Rb_o[:], rb2[:])

        cfac = small.tile([BSH, 1], F32, tag="cfac")
        nc.vector.tensor_scalar_mul(cfac[:], rs[:], LEAK)
        r2t = small.tile([BSH, 1], F32, tag="r2t")
        nc.vector.tensor_scalar_add(r2t[:], cfac[:], 1.0)
        nc.gpsimd.dma_start(r_o[:], r2t[:])
        rec = small.tile([BSH, 1], F32, tag="rec")
        nc.vector.reciprocal(rec[:], r2t[:])
        ratio = small.tile([BSH, 1], F32, tag="ratio")
        nc.vector.tensor_mul(ratio[:], cfac[:], rec[:])
        g1 = small.tile([BSH, DOUT], F32, tag="scr", bufs=4)
        nc.vector.tensor_scalar(g1[:], gbs[:], ratio[:, 0:1], None, ALU.mult)
        onem = small.tile([BSH, 1], F32, tag="onem")
        nc.vector.tensor_scalar(onem[:], ratio[:], -1.0, 1.0, ALU.mult, ALU.add)
        g2 = small.tile([BSH, DOUT], F32, tag="scr", bufs=4)
        nc.vector.tensor_scalar(g2[:], sg[:], onem[:, 0:1], BETA, ALU.mult, ALU.mult)
        gb2 = small.tile([BSH, DOUT], F32, tag="scr", bufs=4)
        nc.vector.tensor_add(gb2[:], g1[:], g2[:])
        nc.gpsimd.dma_start(gb_o[:], gb2[:])

        # ---------------- main loop: 32 big (128,1024) tiles ----------------
        for i in range(BSH):
            for k in range(NK):
                ew_t = big_in.tile([128, DOUT], F32, tag="ew")
                nc.sync.dma_start(ew_t[:], EW[i, bass.ts(k, 128), :])
                rh_t = big_in.tile([128, DOUT], F32, tag="rh")
                nc.sync.dma_start(rh_t[:], RW[i, bass.ts(k, 128), :])

                col = k * BSH + i
                e2_t = big_out.tile([128, DOUT], F32, tag="e2")
                nc.scalar.activation(e2_t[:], ew_t[:], ACTF.Identity,
                                     bias=xc1[:, col:col + 1], scale=BETA * BETA)
                nc.gpsimd.dma_start(EW_o[i, bass.ts(k, 128), :], e2_t[:])

                v_t = big_out.tile([128, DOUT], F32, tag="v")
                nc.vector.scalar_tensor_tensor(v_t[:], rh_t[:], BETA, e2_t[:],
                                               ALU.mult, ALU.add)
                r2_t = big_out.tile([128, DOUT], F32, tag="r2w")
                nc.vector.tensor_mul(r2_t[:], v_t[:], sgrep[:, bass.ts(i, DOUT)])
                nc.gpsimd.dma_start(RW_o[i, bass.ts(k, 128), :], r2_t[:])

    nc.compile()
    return nc


_CACHE = {}


def get_program():
    if "nc" not in _CACHE:
        _CACHE["nc"] = build_program()
    return _CACHE["nc"]


def make_in_maps(x, W, b, u, E_W, E_b, Rh_W, Rh_b, g_bar, r):
    f = lambda a: np.ascontiguousarray(np.asarray(a, dtype=np.float32))
    in_maps = []
    for c in range(NCORES):
        sl = slice(c * BSH, (c + 1) * BSH)
        in_maps.append({
            "x": f(x[sl]),
            "W": f(W),
            "b": f(b).reshape(1, DOUT),
            "u": f(u[sl]),
            "E_W": f(E_W[sl]),
            "E_b": f(E_b[sl]),
            "Rh_W": f(Rh_W[sl]),
            "Rh_b": f(Rh_b[sl]),
            "g_bar": f(g_bar[sl]),
            "r": f(r[sl]),
        })
    return in_maps


def gather_outputs(results):
    cat = lambda name: np.concatenate([results[c][name] for c in range(NCORES)], 0)
    return (cat("s_out"), cat("E_W2"), cat("E_b2"), cat("Rh_W2"),
            cat("Rh_b2"), cat("g_bar2"), cat("r2"))


def kernel(x, W, b, u, E_W, E_b, Rh_W, Rh_b, g_bar, r):
    nc = get_program()
    in_maps = make_in_maps(x, W, b, u, E_W, E_b, Rh_W, Rh_b, g_bar, r)
    res = run_bass_kernel_spmd(nc, in_maps, list(range(NCORES)))
    return gather_outputs(res.results)


# revision 12
# speedup vs baseline: 1.0270x; 1.0270x over previous
"""Trainium2 Bass kernel for nn_DenseOTPE (B=32, DIN=DOUT=1024, 8 cores).

Data-parallel over batch axis 0: each of the 8 NeuronCores processes 4
samples. Params W, b are replicated. No collectives needed (all outputs
are per-sample).

Math (closed form of the reference, per sample i):
    h      = x @ W + b
    u_new  = BETA*u + h
    s      = sigmoid(u_new - THR)
    sg     = s*(1-s)
    E_W2   = BETA^2*E_W + (1+BETA)*x[:,None]            (broadcast over DOUT)
    Rh_W2  = sg[None,:] * (BETA*Rh_W + E_W2)
    E_b2   = BETA^2*E_b + (1+BETA)
    Rh_b2  = sg * (BETA*Rh_b + E_b2)
    ratio  = LEAK*r ; r2 = ratio+1 ; ratio' = ratio/r2
    g_bar2 = ratio'*g_bar + (1-ratio')*BETA*sg
"""

import os
import sys
from contextlib import ExitStack

for _p in ("/opt/trn_rl_repo", "/root/.axon_site/_ro/trn_rl_repo"):
    if os.path.isdir(_p) and _p not in sys.path:
        sys.path.insert(0, _p)

import numpy as np

import concourse.bacc as bacc
import concourse.bass as bass
import concourse.tile as tile
from concourse import mybir
from concourse._compat import get_trn_type
from concourse.bass_utils import run_bass_kernel_spmd

BETA = 0.9
THR = 1.0
LEAK = 0.9
B, DIN, DOUT = 32, 1024, 1024
NCORES = 8
BSH = B // NCORES          # 4 samples per core
NK = DIN // 128            # 8 din chunks of 128 partitions
F32 = mybir.dt.float32
ALU = mybir.AluOpType
ACTF = mybir.ActivationFunctionType


def build_program():
    nc = bacc.Bacc(get_trn_type() or "TRN2", target_bir_lowering=False, debug=False)

    x = nc.dram_tensor("x", [BSH, DIN], F32, kind="ExternalInput").ap()
    W = nc.dram_tensor("W", [DIN, DOUT], F32, kind="ExternalInput").ap()
    b = nc.dram_tensor("b", [1, DOUT], F32, kind="ExternalInput").ap()
    u = nc.dram_tensor("u", [BSH, DOUT], F32, kind="ExternalInput").ap()
    EW = nc.dram_tensor("E_W", [BSH, DIN, DOUT], F32, kind="ExternalInput").ap()
    Eb = nc.dram_tensor("E_b", [BSH, DOUT], F32, kind="ExternalInput").ap()
    RW = nc.dram_tensor("Rh_W", [BSH, DIN, DOUT], F32, kind="ExternalInput").ap()
    Rb = nc.dram_tensor("Rh_b", [BSH, DOUT], F32, kind="ExternalInput").ap()
    gb = nc.dram_tensor("g_bar", [BSH, DOUT], F32, kind="ExternalInput").ap()
    r = nc.dram_tensor("r", [BSH, 1], F32, kind="ExternalInput").ap()

    s_o = nc.dram_tensor("s_out", [BSH, DOUT], F32, kind="ExternalOutput").ap()
    EW_o = nc.dram_tensor("E_W2", [BSH, DIN, DOUT], F32, kind="ExternalOutput").ap()
    Eb_o = nc.dram_tensor("E_b2", [BSH, DOUT], F32, kind="ExternalOutput").ap()
    RW_o = nc.dram_tensor("Rh_W2", [BSH, DIN, DOUT], F32, kind="ExternalOutput").ap()
    Rb_o = nc.dram_tensor("Rh_b2", [BSH, DOUT], F32, kind="ExternalOutput").ap()
    gb_o = nc.dram_tensor("g_bar2", [BSH, DOUT], F32, kind="ExternalOutput").ap()
    r_o = nc.dram_tensor("r2", [BSH, 1], F32, kind="ExternalOutput").ap()

    with tile.TileContext(nc) as tc, ExitStack() as ctx:
        small = ctx.enter_context(tc.tile_pool(name="small", bufs=1))
        const = ctx.enter_context(tc.tile_pool(name="const", bufs=1))
        wpool = ctx.enter_context(tc.tile_pool(name="wpool", bufs=1))
        psum = ctx.enter_context(tc.tile_pool(name="psum", bufs=2, space="PSUM"))
        big_in = ctx.enter_context(tc.tile_pool(name="big_in", bufs=4))
        big_out = ctx.enter_context(tc.tile_pool(name="big_out", bufs=3))

        # ---------------- prologue: small loads ----------------
        xs = small.tile([BSH, DIN], F32, tag="xs")
        nc.sync.dma_start(xs[:], x[:])
        us = small.tile([BSH, DOUT], F32, tag="us")
        nc.sync.dma_start(us[:], u[:])
        ebs = small.tile([BSH, DOUT], F32, tag="ebs")
        nc.sync.dma_start(ebs[:], Eb[:])
        rbs = small.tile([BSH, DOUT], F32, tag="rbs")
        nc.sync.dma_start(rbs[:], Rb[:])
        gbs = small.tile([BSH, DOUT], F32, tag="gbs")
        nc.sync.dma_start(gbs[:], gb[:])
        rs = small.tile([BSH, 1], F32, tag="rs")
        nc.sync.dma_start(rs[:], r[:])
        b_row = small.tile([1, DOUT], F32, tag="brow")
        nc.sync.dma_start(b_row[:], b[:])

        w_sb = wpool.tile([128, NK * DOUT], F32, tag="w")
        for k in range(NK):
            nc.sync.dma_start(w_sb[:, bass.ts(k, DOUT)], W[bass.ts(k, 128), :])

        # constants
        ones4 = const.tile([BSH, BSH], F32, tag="ones4")
        nc.vector.memset(ones4[:], 1.0)
        ident4 = const.tile([BSH, BSH], F32, tag="id4")
        nc.gpsimd.affine_select(
            ident4[:], ones4[:], pattern=[[1, BSH]], compare_op=ALU.is_equal,
            fill=0.0, base=0, channel_multiplier=-1,
        )
        bident4 = const.tile([BSH, BSH], F32, tag="bid4")
        nc.vector.tensor_scalar_mul(bident4[:], ident4[:], BETA)
        ones1 = const.tile([1, 128], F32, tag="ones1")
        nc.vector.memset(ones1[:], 1.0)
        negthr = const.tile([BSH, 1], F32, tag="negthr")
        nc.vector.memset(negthr[:], -THR)
        # sel[:, i*128:(i+1)*128]: (BSH,128) matrix with row i all-ones —
        # lhsT that both selects sample i's sg row and replicates it to 128
        # partitions in one matmul.
        ones_sel = const.tile([BSH, BSH * 128], F32, tag="ones_sel")
        nc.vector.memset(ones_sel[:], 1.0)
        sel = const.tile([BSH, BSH * 128], F32, tag="sel")
        nc.gpsimd.affine_select(
            sel[:], ones_sel[:], pattern=[[1, BSH], [0, 128]],
            compare_op=ALU.is_equal, fill=0.0, base=0, channel_multiplier=-1,
        )

        # x^T tiles: xt[:, k*BSH+i] = x[i, k*128+p]; via PE transpose matmul
        xt = const.tile([128, NK * BSH], F32, tag="xt")
        for k in range(NK):
            pt = psum.tile([128, BSH], F32, tag="pxt")
            nc.tensor.matmul(pt[:], xs[:, bass.ts(k, 128)], ident4[:],
                             start=True, stop=True)
            nc.vector.tensor_copy(xt[:, bass.ts(k, BSH)], pt[:])
        xc1 = const.tile([128, NK * BSH], F32, tag="xc1")
        nc.vector.tensor_scalar_mul(xc1[:], xt[:], 1.0 + BETA)

        # u_new = x@W + b + BETA*u accumulated in PSUM; s = sigmoid(u_new-THR)
        s_sb = small.tile([BSH, DOUT], F32, tag="s")
        sg = small.tile([BSH, DOUT], F32, tag="sg")
        for n in range(2):
            ph = psum.tile([BSH, 512], F32, tag="ph")
            for k in range(NK):
                nc.tensor.matmul(
                    ph[:], xt[:, bass.ts(k, BSH)],
                    w_sb[:, k * DOUT + n * 512: k * DOUT + n * 512 + 512],
                    start=(k == 0), stop=False)
            nc.tensor.matmul(ph[:], ones1[0:1, 0:BSH], b_row[0:1, bass.ts(n, 512)],
                             start=False, stop=False)
            nc.tensor.matmul(ph[:], bident4[:], us[:, bass.ts(n, 512)],
                             start=False, stop=True)
            nc.scalar.activation(s_sb[:, bass.ts(n, 512)], ph[:], ACTF.Sigmoid,
                                 bias=negthr[:, 0:1])
        nc.sync.dma_start(s_o[:], s_sb[:])

        # sg = s*(1-s)
        tmp1 = small.tile([BSH, DOUT], F32, tag="scr", bufs=3)
        nc.vector.tensor_scalar(tmp1[:], s_sb[:], -1.0, 1.0, ALU.mult, ALU.add)
        nc.vector.tensor_mul(sg[:], tmp1[:], s_sb[:])

        # sg replicated across 128 partitions, per sample
        sgrep = const.tile([128, BSH * DOUT], F32, tag="sgrep")
        for i in range(BSH):
            for n in range(2):
                pr = psum.tile([128, 512], F32, tag="pr")
                nc.tensor.matmul(pr[:], sel[:, bass.ts(i, 128)],
                                 sg[:, bass.ts(n, 512)], start=True, stop=True)
                nc.vector.tensor_copy(
                    sgrep[:, i * DOUT + n * 512: i * DOUT + n * 512 + 512], pr[:])

        # ---------------- small trace updates ----------------
        eb2 = small.tile([BSH, DOUT], F32, tag="scr", bufs=3)
        nc.vector.tensor_scalar(eb2[:], ebs[:], BETA * BETA, 1.0 + BETA,
                                ALU.mult, ALU.add)
        nc.gpsimd.dma_start(Eb_o[:], eb2[:])
        vb = small.tile([BSH, DOUT], F32, tag="scr", bufs=3)
        nc.vector.scalar_tensor_tensor(vb[:], rbs[:], BETA, eb2[:],
                                       ALU.mult, ALU.add)
        rb2 = small.tile([BSH, DOUT], F32, tag="scr", bufs=3)
        nc.vector.tensor_mul(rb2[:], vb[:], sg[:])
        nc.gpsimd.dma_start(Rb_o[:], rb2[:])

        cfac = small.tile([BSH, 1], F32, tag="cfac")
        nc.vector.tensor_scalar_mul(cfac[:], rs[:], LEAK)
        r2t = small.tile([BSH, 1], F32, tag="r2t")
        nc.vector.tensor_scalar_add(r2t[:], cfac[:], 1.0)
        nc.gpsimd.dma_start(r_o[:], r2t[:])
        rec = small.tile([BSH, 1], F32, tag="rec")
        nc.vector.reciprocal(rec[:], r2t[:])
        ratio = small.tile([BSH, 1], F32, tag="ratio")
        nc.vector.tensor_mul(ratio[:], cfac[:], rec[:])
        g1 = small.tile([BSH, DOUT], F32, tag="scr", bufs=3)
        nc.vector.tensor_scalar(g1[:], gbs[:], ratio[:, 0:1], None, ALU.mult)
        onem = small.tile([BSH, 1], F32, tag="onem")
        nc.vector.tensor_scalar(onem[:], ratio[:], -1.0, 1.0, ALU.mult, ALU.add)
        g2 = small.tile([BSH, DOUT], F32, tag="scr", bufs=3)
        nc.vector.tensor_scalar(g2[:], sg[:], onem[:, 0:1], BETA, ALU.mult, ALU.mult)
        gb2 = small.tile([BSH, DOUT], F32, tag="scr", bufs=3)
        nc.vector.tensor_add(gb2[:], g1[:], g2[:])
        nc.gpsimd.dma_start(gb_o[:], gb2[:])

        # ---------------- main loop: 32 big (128,1024) tiles ----------------
        for i in range(BSH):
            for k in range(NK):
                ew_t = big_in.tile([128, DOUT], F32, tag="ew")
                nc.sync.dma_start(ew_t[:], EW[i, bass.ts(k, 128), :])
                rh_t = big_in.tile([128, DOUT], F32, tag="rh")
                nc.sync.dma_start(rh_t[:], RW[i, bass.ts(k, 128), :])

                col = k * BSH + i
                e2_t = big_out.tile([128, DOUT], F32, tag="e2", bufs=4)
                nc.scalar.activation(e2_t[:], ew_t[:], ACTF.Identity,
                                     bias=xc1[:, col:col + 1], scale=BETA * BETA)
                nc.gpsimd.dma_start(EW_o[i, bass.ts(k, 128), :], e2_t[:])

                v_t = big_out.tile([128, DOUT], F32, tag="v", bufs=6)
                nc.vector.scalar_tensor_tensor(v_t[:], rh_t[:], BETA, e2_t[:],
                                               ALU.mult, ALU.add)
                r2_t = big_out.tile([128, DOUT], F32, tag="r2w", bufs=3)
                nc.vector.tensor_mul(r2_t[:], v_t[:], sgrep[:, bass.ts(i, DOUT)])
                nc.gpsimd.dma_start(RW_o[i, bass.ts(k, 128), :], r2_t[:])

    nc.compile()
    return nc


_CACHE = {}


def get_program():
    if "nc" not in _CACHE:
        _CACHE["nc"] = build_program()
    return _CACHE["nc"]


def make_in_maps(x, W, b, u, E_W, E_b, Rh_W, Rh_b, g_bar, r):
    f = lambda a: np.ascontiguousarray(np.asarray(a, dtype=np.float32))
    in_maps = []
    for c in range(NCORES):
        sl = slice(c * BSH, (c + 1) * BSH)
        in_maps.append({
            "x": f(x[sl]),
            "W": f(W),
            "b": f(b).reshape(1, DOUT),
            "u": f(u[sl]),
            "E_W": f(E_W[sl]),
            "E_b": f(E_b[sl]),
            "Rh_W": f(Rh_W[sl]),
            "Rh_b": f(Rh_b[sl]),
            "g_bar": f(g_bar[sl]),
            "r": f(r[sl]),
        })
    return in_maps


def gather_outputs(results):
    cat = lambda name: np.concatenate([results[c][name] for c in range(NCORES)], 0)
    return (cat("s_out"), cat("E_W2"), cat("E_b2"), cat("Rh_W2"),
            cat("Rh_b2"), cat("g_bar2"), cat("r2"))


def kernel(x, W, b, u, E_W, E_b, Rh_W, Rh_b, g_bar, r):
    nc = get_program()
    in_maps = make_in_maps(x, W, b, u, E_W, E_b, Rh_W, Rh_b, g_bar, r)
    res = run_bass_kernel_spmd(nc, in_maps, list(range(NCORES)))
    return gather_outputs(res.results)
